# revision 38
# baseline (speedup 1.0000x reference)
"""Trainium2 Bass kernel for the all-pairs spring-energy sum (EnergyLossVectorized).

Contract: kernel(**inputs) takes FULL unsharded inputs (p [32768,2] f32,
edge_attr [E,2] f32, src/dst [E] i32 with E = 64*512*511), returns the FULL
scalar output, distributing across 8 NeuronCores internally.

Strategy: src/dst produced by the reference's setup_inputs() are the
deterministic all-directed-pairs (i != j) indices per graph, in i-major
order.  We verify that structure on the host (falling back to a straight
numpy evaluation if it ever doesn't hold) and then compute the energy with
a gather-free formulation:

  For each graph g (512 nodes), the 512x512 grid D2[i,j] = |p_i - p_j|^2 is
  computed on the tensor engine as a K=8 matmul  D2 = PL^T @ PR with
     PL features: [ x,  y,  rhi, rmid, rlo, 1, 1, 1 ]
     PR features: [-2x, -2y, 1,   1,   1,   rhi, rmid, rlo ]
  where r = x^2 + y^2 of the bf16-rounded coords is carried as three bf16
  limbs, so the PSUM result equals |p_i - p_j|^2 to ~fp32 accuracy (no
  cancellation blowup), guaranteeing D2 >= -1e-5 and sqrt(D2+EPS) NaN-free.

  edge_attr (l, k) is re-laid-out on the host into per-graph [512,512] bf16
  grids with k=0 on the diagonal, interleaved [p, {l,k}, t, j] so each
  graph is a single contiguous 1 MB DMA.  Per half-graph tile [128 x 1024]:
     s  = sqrt(D2 + EPS)            (scalar engine, PSUM -> SBUF bf16)
     u  = s - l                     (DVE / GPSIMD)
     e  = (u ^ 2) * k  + row-sum    (DVE scalar_tensor_tensor, accum_out)
  Per-row partials accumulate in parts[128, 16]; the final reduction is one
  tensor_reduce + a [1x1] ones-matmul + 0.5 scale on device; the host sums
  the 8 per-core scalars.

Memory traffic per core: 8 graphs * 1 MB = 8.4 MB bf16 -> ~24 us roofline.
"""

import os
import sys

import numpy as np

for _p in ("/opt/trn_rl_repo", "/root/.axon_site/_ro/trn_rl_repo"):
    if os.path.isdir(_p) and _p not in sys.path:
        sys.path.insert(0, _p)

import ml_dtypes

bf16 = ml_dtypes.bfloat16

NUM_GRAPHS = 64
N = 512                      # nodes per graph
NCORES = 8
GPC = NUM_GRAPHS // NCORES   # graphs per core = 8
PB = 128                     # partition block (i-tile)
EPS = 1e-5                   # sqrt clamp; D2 >= -1e-5 guaranteed by 3-limb r

# per-tile ([128,512], 32 tiles) engine assignment, tuned from HW probes:
# DVE TT bf16 hits 2x mode only at free-dim <= 512 (~380ns); tensor_scalar
# with accum_out runs 4x (~330ns); GPS TT ~1.1us; ACT ~687ns; matmul ~600ns.
# SUB (u=s-l): D=vector, G=gpsimd
# VAR: A = ACT Square(u) then DVE e=u2*k ; B = DVE v=u*k then DVE e=v*u
# RED: P = PE ones-matmul into PSUM acc ; T = DVE tensor_scalar accum
SUB_PAT = "GDGDGDGDGDGDGDGDGDGDGDGDGDGDGDGD"
VAR_PAT = "AAAAAAABBBBBBBBBAAAAAAABBBBBBBBB"
RED_PAT = "TTTTTTTPPPPPPPPPTTTTTTTPPPPPPPPP"


def _build_nc(gpc=GPC, n=N, pb=PB, debug=False):
    """Build + compile the per-core Bass program (SPMD, same on all cores)."""
    import concourse.bass as bass
    import concourse.tile as tile
    from concourse import bacc, mybir

    tb = n // pb             # i-tiles per graph (4)
    th = tb // 2             # halves per graph (2), each [pb, 2, n]
    fdt = mybir.dt.float32
    bdt = mybir.dt.bfloat16
    AF = mybir.ActivationFunctionType
    AL = mybir.AluOpType

    nc = bacc.Bacc("TRN2", target_bir_lowering=False, debug=debug,
                   num_devices=NCORES)

    # lk: [graph, partition, {l,k}, t*j] so one graph = 1 contiguous DMA
    lk_d = nc.dram_tensor("lk", [gpc, pb, 2, tb * n], bdt,
                          kind="ExternalInput")
    pl_d = nc.dram_tensor("plin", [64, 4 * n], bdt, kind="ExternalInput")
    pr_d = nc.dram_tensor("prin", [64, 4 * n], bdt, kind="ExternalInput")
    out_d = nc.dram_tensor("out", [1, 1], fdt, kind="ExternalOutput")

    lk = lk_d.ap()

    with tile.TileContext(nc) as tc:
        with (
            tc.tile_pool(name="const", bufs=1) as const,
            tc.tile_pool(name="lkp", bufs=3) as lkp,
            tc.tile_pool(name="work", bufs=6) as work,
            tc.tile_pool(name="psum", bufs=6, space="PSUM") as psum,
            tc.tile_pool(name="accp", bufs=1, space="PSUM") as accp,
        ):
            # host-precomputed matmul operands (see _build_plt_prt)
            plt = const.tile([64, 4 * n], bdt)
            prt = const.tile([64, 4 * n], bdt)
            nc.sync.dma_start(plt[:], pl_d.ap())
            nc.sync.dma_start(prt[:], pr_d.ap())

            ones_col = const.tile([pb, 1], fdt)
            nc.vector.memset(ones_col[:], 1.0)
            ones_bf = const.tile([pb, 1], bdt)
            nc.vector.memset(ones_bf[:], 1.0)
            eps_col = const.tile([pb, 1], fdt)
            nc.vector.memset(eps_col[:], EPS)
            zero_col = const.tile([pb, 1], fdt)
            nc.vector.memset(zero_col[:], 0.0)

            n_tiles = gpc * tb
            n_ts = RED_PAT[:n_tiles].count("T")
            parts = const.tile([pb, max(1, n_ts)], fdt)
            acc = accp.tile([1, n], fdt)
            n_pe = RED_PAT[:n_tiles].count("P")

            idx = 0
            ts_i = 0
            pe_i = 0
            for g2 in range(gpc // 2):      # 2-graph DMA chunks (2 MB)
                lkt = lkp.tile([pb, 2, 2, tb * n], bdt)
                nc.sync.dma_start(
                    lkt[:],
                    lk[2 * g2:2 * g2 + 2].rearrange("g p c w -> p g c w"))
                for gg2 in range(2):
                    g = 2 * g2 + gg2
                    g_, gg = divmod(g, 4)
                    for t in range(tb):
                        ps = psum.tile([pb, n], fdt)
                        nc.tensor.matmul(
                            ps[:],
                            plt[32 * g_:32 * g_ + 8,
                                gg * n + t * pb: gg * n + (t + 1) * pb],
                            prt[32 * g_:32 * g_ + 8, gg * n:(gg + 1) * n],
                            start=True, stop=True,
                        )
                        s = work.tile([pb, n], bdt, tag="s")
                        nc.scalar.activation(s[:], ps[:], AF.Sqrt,
                                             bias=eps_col[:])
                        u = work.tile([pb, n], bdt, tag="u")
                        lsl = lkt[:, gg2, 0, t * n:(t + 1) * n]
                        ksl = lkt[:, gg2, 1, t * n:(t + 1) * n]
                        if SUB_PAT[idx] == "G":
                            nc.gpsimd.tensor_sub(u[:], s[:], lsl)
                        else:
                            nc.vector.tensor_sub(u[:], s[:], lsl)
                        v = work.tile([pb, n], bdt, tag="v")
                        e = work.tile([pb, n], bdt, tag="e")
                        if VAR_PAT[idx] == "A":
                            nc.scalar.activation(v[:], u[:], AF.Square,
                                                 bias=zero_col[:])
                            nc.vector.tensor_mul(e[:], v[:], ksl)
                        else:
                            nc.vector.tensor_mul(v[:], u[:], ksl)
                            nc.vector.tensor_mul(e[:], v[:], u[:])
                        if RED_PAT[idx] == "P":
                            nc.tensor.matmul(
                                acc[:], ones_bf[:], e[:],
                                start=(pe_i == 0), stop=(pe_i == n_pe - 1),
                                skip_group_check=True,
                            )
                            pe_i += 1
                        else:
                            e2 = work.tile([pb, n], bdt, tag="e2")
                            nc.vector.tensor_scalar(
                                e2[:], e[:], 1.0, 0.0, AL.mult, AL.add,
                                accum_out=parts[:, ts_i:ts_i + 1])
                            ts_i += 1
                        idx += 1

            # ---- final reduction to a scalar ----
            # parts [pb, n_ts] -> [pb,1] ; acc [1,n] -> add col-sums
            pr1 = const.tile([pb, 1], fdt)
            nc.vector.tensor_reduce(
                pr1[:], parts[:], axis=mybir.AxisListType.X, op=AL.add)
            acc_sb = const.tile([1, n], fdt)
            nc.vector.tensor_copy(acc_sb[:], acc[:])
            acc1 = const.tile([1, 1], fdt)
            nc.vector.tensor_reduce(
                acc1[:], acc_sb[:], axis=mybir.AxisListType.X, op=AL.add)
            acc11 = accp.tile([1, 1], fdt)
            nc.tensor.matmul(acc11[:], ones_col[:], pr1[:],
                             start=True, stop=True, skip_group_check=True)
            acc11_sb = const.tile([1, 1], fdt)
            nc.vector.tensor_copy(acc11_sb[:], acc11[:])
            tot = const.tile([1, 1], fdt)
            nc.vector.tensor_add(tot[:], acc11_sb[:], acc1[:])
            tot2 = const.tile([1, 1], fdt)
            nc.vector.tensor_scalar_mul(tot2[:], tot[:], 0.5)
            nc.sync.dma_start(out_d.ap(), tot2[:])

    nc.compile()
    return nc


_NC_CACHE = {}


def _get_nc(gpc=GPC, n=N, pb=PB):
    key = (gpc, n, pb)
    if key not in _NC_CACHE:
        _NC_CACHE[key] = _build_nc(gpc, n, pb)
    return _NC_CACHE[key]


def _expected_pairs(num_graphs, n):
    i = np.repeat(np.arange(n, dtype=np.int64), n)
    j = np.tile(np.arange(n, dtype=np.int64), n)
    keep = i != j
    si, sj = i[keep], j[keep]
    off = (np.arange(num_graphs, dtype=np.int64) * n)[:, None]
    src = (off + si[None, :]).reshape(-1)
    dst = (off + sj[None, :]).reshape(-1)
    return src.astype(np.int32), dst.astype(np.int32)


def _structure_ok(src, dst):
    if src.shape != (NUM_GRAPHS * N * (N - 1),):
        return False
    esrc, edst = _expected_pairs(NUM_GRAPHS, N)
    return np.array_equal(src, esrc) and np.array_equal(dst, edst)


def _fallback_numpy(p, edge_attr, src, dst):
    start = p[src].astype(np.float64)
    end = p[dst].astype(np.float64)
    t12 = ((start - end) ** 2).sum(axis=1)
    l = edge_attr[:, 0].astype(np.float64)
    k = edge_attr[:, 1].astype(np.float64)
    energy = k / 2.0 * (t12 + l * l - 2.0 * l * np.sqrt(t12))
    return np.float32(energy.sum())


def _build_plt_prt(p_core, gpc=GPC, n=N):
    """p_core [gpc*n, 2] f32 -> (plt, prt) [64, 4n] bf16 matmul operands."""
    xb = p_core.reshape(gpc, n, 2).astype(bf16)          # bf16-rounded coords
    xf = xb[..., 0].astype(np.float32)
    yf = xb[..., 1].astype(np.float32)
    r = xf * xf + yf * yf
    rhi = r.astype(bf16)
    r1 = r - rhi.astype(np.float32)
    rmid = r1.astype(bf16)
    r2 = r1 - rmid.astype(np.float32)
    rlo = r2.astype(bf16)
    plt = np.ones((64, 4 * n), dtype=bf16)
    prt = np.ones((64, 4 * n), dtype=bf16)
    feats_l = [xb[..., 0], xb[..., 1], rhi, rmid, rlo]
    feats_r = [(xb[..., 0] * bf16(-2.0)), (xb[..., 1] * bf16(-2.0)),
               None, None, None, rhi, rmid, rlo]
    for g in range(gpc):
        g_, gg = divmod(g, 4)
        cols = slice(gg * n, (gg + 1) * n)
        for f, arr in enumerate(feats_l):
            plt[32 * g_ + f, cols] = arr[g]
        for f, arr in enumerate(feats_r):
            if arr is not None:
                prt[32 * g_ + f, cols] = arr[g]
    return plt, prt


def _build_grids(edge_attr):
    """edge_attr [E,2] f32 -> lk bf16 array [NCORES, GPC, PB, 2, TB, N]."""
    tb = N // PB
    ea = edge_attr.astype(bf16).reshape(NUM_GRAPHS, N * (N - 1), 2)
    offdiag = (~np.eye(N, dtype=bool)).reshape(-1)
    grid = np.zeros((2, NUM_GRAPHS, N * N), dtype=bf16)
    grid[0][:, offdiag] = ea[:, :, 0]
    grid[1][:, offdiag] = ea[:, :, 1]
    # [2, graphs, t, p, j] -> [cores, gpc, p, 2, t*j]
    g5 = grid.reshape(2, NUM_GRAPHS, tb, PB, N)
    lk = np.ascontiguousarray(g5.transpose(1, 3, 0, 2, 4))  # [G, PB, 2, tb, N]
    return lk.reshape(NCORES, GPC, PB, 2, tb * N)


def kernel(p, edge_attr, src, dst):
    p = np.ascontiguousarray(np.asarray(p, dtype=np.float32))
    edge_attr = np.ascontiguousarray(np.asarray(edge_attr, dtype=np.float32))
    src = np.asarray(src, dtype=np.int32)
    dst = np.asarray(dst, dtype=np.int32)

    if not _structure_ok(src, dst):
        return _fallback_numpy(p, edge_attr, src, dst)

    from concourse.bass_utils import run_bass_kernel_spmd

    lk = _build_grids(edge_attr)
    pcs = p.reshape(NCORES, GPC * N, 2)

    nc = _get_nc()
    in_maps = []
    for c in range(NCORES):
        plt, prt = _build_plt_prt(pcs[c])
        in_maps.append({"lk": lk[c], "plin": plt, "prin": prt})
    res = run_bass_kernel_spmd(nc, in_maps, list(range(NCORES)))
    total = sum(float(res.results[c]["out"][0, 0]) for c in range(NCORES))
    return np.float32(total)


if __name__ == "__main__":
    nc = _get_nc()
    print("compiled ok")


# revision 40
# speedup vs baseline: 1.0279x; 1.0279x over previous
"""Trainium2 Bass kernel for the all-pairs spring-energy sum (EnergyLossVectorized).

Contract: kernel(**inputs) takes FULL unsharded inputs (p [32768,2] f32,
edge_attr [E,2] f32, src/dst [E] i32 with E = 64*512*511), returns the FULL
scalar output, distributing across 8 NeuronCores internally.

Strategy: src/dst produced by the reference's setup_inputs() are the
deterministic all-directed-pairs (i != j) indices per graph, in i-major
order.  We verify that structure on the host (falling back to a straight
numpy evaluation if it ever doesn't hold) and then compute the energy with
a gather-free formulation:

  For each graph g (512 nodes), the 512x512 grid D2[i,j] = |p_i - p_j|^2 is
  computed on the tensor engine as a K=8 matmul  D2 = PL^T @ PR with
     PL features: [ x,  y,  rhi, rmid, rlo, 1, 1, 1 ]
     PR features: [-2x, -2y, 1,   1,   1,   rhi, rmid, rlo ]
  where r = x^2 + y^2 of the bf16-rounded coords is carried as three bf16
  limbs, so the PSUM result equals |p_i - p_j|^2 to ~fp32 accuracy (no
  cancellation blowup), guaranteeing D2 >= -1e-5 and sqrt(D2+EPS) NaN-free.

  edge_attr (l, k) is re-laid-out on the host into per-graph [512,512] bf16
  grids with k=0 on the diagonal, interleaved [p, {l,k}, t, j] so each
  graph is a single contiguous 1 MB DMA.  Per half-graph tile [128 x 1024]:
     s  = sqrt(D2 + EPS)            (scalar engine, PSUM -> SBUF bf16)
     u  = s - l                     (DVE / GPSIMD)
     e  = (u ^ 2) * k  + row-sum    (DVE scalar_tensor_tensor, accum_out)
  Per-row partials accumulate in parts[128, 16]; the final reduction is one
  tensor_reduce + a [1x1] ones-matmul + 0.5 scale on device; the host sums
  the 8 per-core scalars.

Memory traffic per core: 8 graphs * 1 MB = 8.4 MB bf16 -> ~24 us roofline.
"""

import os
import sys

import numpy as np

for _p in ("/opt/trn_rl_repo", "/root/.axon_site/_ro/trn_rl_repo"):
    if os.path.isdir(_p) and _p not in sys.path:
        sys.path.insert(0, _p)

import ml_dtypes

bf16 = ml_dtypes.bfloat16

NUM_GRAPHS = 64
N = 512                      # nodes per graph
NCORES = 8
GPC = NUM_GRAPHS // NCORES   # graphs per core = 8
PB = 128                     # partition block (i-tile)
EPS = 1e-5                   # sqrt clamp; D2 >= -1e-5 guaranteed by 3-limb r

# per-tile ([128,512], 32 tiles) engine assignment, tuned from HW probes:
# DVE TT bf16 hits 2x mode only at free-dim <= 512 (~380ns); tensor_scalar
# with accum_out runs 4x (~330ns); GPS TT ~1.1us; ACT ~687ns; matmul ~600ns.
# SUB (u=s-l): D=vector, G=gpsimd
# VAR: A = ACT Square(u) then DVE e=u2*k ; B = DVE v=u*k then DVE e=v*u
# RED: P = PE ones-matmul into PSUM acc ; T = DVE tensor_scalar accum
VAR_PAT = "ABABABABABABABABABABABABABABABAB"   # A: sub=D; B: sub=G
RED_PAT = "TPPTPPTPPTPPTPPTPPTPPTPPTPPTPPTP"   # P=21, T=11


def _build_nc(gpc=GPC, n=N, pb=PB, debug=False):
    """Build + compile the per-core Bass program (SPMD, same on all cores)."""
    import concourse.bass as bass
    import concourse.tile as tile
    from concourse import bacc, mybir

    tb = n // pb             # i-tiles per graph (4)
    th = tb // 2             # halves per graph (2), each [pb, 2, n]
    fdt = mybir.dt.float32
    bdt = mybir.dt.bfloat16
    AF = mybir.ActivationFunctionType
    AL = mybir.AluOpType

    nc = bacc.Bacc("TRN2", target_bir_lowering=False, debug=debug,
                   num_devices=NCORES)

    # lk: [graph, partition, {l,k}, t*j] so one graph = 1 contiguous DMA
    lk_d = nc.dram_tensor("lk", [gpc, pb, 2, tb * n], bdt,
                          kind="ExternalInput")
    pl_d = nc.dram_tensor("plin", [64, 4 * n], bdt, kind="ExternalInput")
    pr_d = nc.dram_tensor("prin", [64, 4 * n], bdt, kind="ExternalInput")
    out_d = nc.dram_tensor("out", [1, 1], fdt, kind="ExternalOutput")

    lk = lk_d.ap()

    with tile.TileContext(nc) as tc:
        with (
            tc.tile_pool(name="const", bufs=1) as const,
            tc.tile_pool(name="lkp", bufs=3) as lkp,
            tc.tile_pool(name="work", bufs=6) as work,
            tc.tile_pool(name="psum", bufs=6, space="PSUM") as psum,
            tc.tile_pool(name="accp", bufs=1, space="PSUM") as accp,
        ):
            # host-precomputed matmul operands (see _build_plt_prt)
            plt = const.tile([64, 4 * n], bdt)
            prt = const.tile([64, 4 * n], bdt)
            nc.sync.dma_start(plt[:], pl_d.ap())
            nc.sync.dma_start(prt[:], pr_d.ap())

            ones_col = const.tile([pb, 1], fdt)
            nc.vector.memset(ones_col[:], 1.0)
            ones_bf = const.tile([pb, 1], bdt)
            nc.vector.memset(ones_bf[:], 1.0)
            eps_col = const.tile([pb, 1], fdt)
            nc.vector.memset(eps_col[:], EPS)
            zero_col = const.tile([pb, 1], fdt)
            nc.vector.memset(zero_col[:], 0.0)

            n_tiles = gpc * tb
            n_ts = RED_PAT[:n_tiles].count("T")
            parts = const.tile([pb, max(1, n_ts)], fdt)
            acc = accp.tile([1, n], fdt)
            n_pe = RED_PAT[:n_tiles].count("P")

            idx = 0
            ts_i = 0
            pe_i = 0
            for g2 in range(gpc // 2):      # 2-graph DMA chunks (2 MB)
                lkt = lkp.tile([pb, 4 * tb * n], bdt)
                nc.sync.dma_start(
                    lkt[:],
                    lk[2 * g2:2 * g2 + 2].rearrange("g p c w -> p g c w"))
                for gg2 in range(2):
                    g = 2 * g2 + gg2
                    g_, gg = divmod(g, 4)
                    for t in range(tb):
                        ps = psum.tile([pb, n], fdt)
                        nc.tensor.matmul(
                            ps[:],
                            plt[32 * g_:32 * g_ + 8,
                                gg * n + t * pb: gg * n + (t + 1) * pb],
                            prt[32 * g_:32 * g_ + 8, gg * n:(gg + 1) * n],
                            start=True, stop=True,
                        )
                        s = work.tile([pb, n], bdt, tag="s")
                        nc.scalar.activation(s[:], ps[:], AF.Sqrt,
                                             bias=eps_col[:])
                        u = work.tile([pb, n], bdt, tag="u")
                        lo = (2 * gg2 + 0) * tb * n + t * n
                        ko = (2 * gg2 + 1) * tb * n + t * n
                        lsl = lkt[:, lo:lo + n]
                        ksl = lkt[:, ko:ko + n]
                        v = work.tile([pb, n], bdt, tag="v")
                        e = work.tile([pb, n], bdt, tag="e")
                        if VAR_PAT[idx] == "A":
                            nc.vector.tensor_sub(u[:], s[:], lsl)
                            nc.scalar.activation(v[:], u[:], AF.Square,
                                                 bias=zero_col[:])
                            nc.vector.tensor_mul(e[:], v[:], ksl)
                        else:
                            nc.gpsimd.tensor_sub(u[:], s[:], lsl)
                            nc.vector.tensor_mul(v[:], u[:], ksl)
                            nc.vector.tensor_mul(e[:], v[:], u[:])
                        if RED_PAT[idx] == "P":
                            nc.tensor.matmul(
                                acc[:], ones_bf[:], e[:],
                                start=(pe_i == 0), stop=(pe_i == n_pe - 1),
                                skip_group_check=True,
                            )
                            pe_i += 1
                        else:
                            e2 = work.tile([pb, n], bdt, tag="e2")
                            nc.vector.tensor_scalar(
                                e2[:], e[:], 1.0, 0.0, AL.mult, AL.add,
                                accum_out=parts[:, ts_i:ts_i + 1])
                            ts_i += 1
                        idx += 1

            # ---- final reduction to a scalar ----
            # parts [pb, n_ts] -> [pb,1] ; acc [1,n] -> add col-sums
            pr1 = const.tile([pb, 1], fdt)
            nc.vector.tensor_reduce(
                pr1[:], parts[:], axis=mybir.AxisListType.X, op=AL.add)
            acc_sb = const.tile([1, n], fdt)
            nc.vector.tensor_copy(acc_sb[:], acc[:])
            acc1 = const.tile([1, 1], fdt)
            nc.vector.tensor_reduce(
                acc1[:], acc_sb[:], axis=mybir.AxisListType.X, op=AL.add)
            acc11 = accp.tile([1, 1], fdt)
            nc.tensor.matmul(acc11[:], ones_col[:], pr1[:],
                             start=True, stop=True, skip_group_check=True)
            acc11_sb = const.tile([1, 1], fdt)
            nc.vector.tensor_copy(acc11_sb[:], acc11[:])
            tot = const.tile([1, 1], fdt)
            nc.vector.tensor_add(tot[:], acc11_sb[:], acc1[:])
            tot2 = const.tile([1, 1], fdt)
            nc.vector.tensor_scalar_mul(tot2[:], tot[:], 0.5)
            nc.sync.dma_start(out_d.ap(), tot2[:])

    nc.compile()
    return nc


_NC_CACHE = {}


def _get_nc(gpc=GPC, n=N, pb=PB):
    key = (gpc, n, pb)
    if key not in _NC_CACHE:
        _NC_CACHE[key] = _build_nc(gpc, n, pb)
    return _NC_CACHE[key]


def _expected_pairs(num_graphs, n):
    i = np.repeat(np.arange(n, dtype=np.int64), n)
    j = np.tile(np.arange(n, dtype=np.int64), n)
    keep = i != j
    si, sj = i[keep], j[keep]
    off = (np.arange(num_graphs, dtype=np.int64) * n)[:, None]
    src = (off + si[None, :]).reshape(-1)
    dst = (off + sj[None, :]).reshape(-1)
    return src.astype(np.int32), dst.astype(np.int32)


def _structure_ok(src, dst):
    if src.shape != (NUM_GRAPHS * N * (N - 1),):
        return False
    esrc, edst = _expected_pairs(NUM_GRAPHS, N)
    return np.array_equal(src, esrc) and np.array_equal(dst, edst)


def _fallback_numpy(p, edge_attr, src, dst):
    start = p[src].astype(np.float64)
    end = p[dst].astype(np.float64)
    t12 = ((start - end) ** 2).sum(axis=1)
    l = edge_attr[:, 0].astype(np.float64)
    k = edge_attr[:, 1].astype(np.float64)
    energy = k / 2.0 * (t12 + l * l - 2.0 * l * np.sqrt(t12))
    return np.float32(energy.sum())


def _build_plt_prt(p_core, gpc=GPC, n=N):
    """p_core [gpc*n, 2] f32 -> (plt, prt) [64, 4n] bf16 matmul operands."""
    xb = p_core.reshape(gpc, n, 2).astype(bf16)          # bf16-rounded coords
    xf = xb[..., 0].astype(np.float32)
    yf = xb[..., 1].astype(np.float32)
    r = xf * xf + yf * yf
    rhi = r.astype(bf16)
    r1 = r - rhi.astype(np.float32)
    rmid = r1.astype(bf16)
    r2 = r1 - rmid.astype(np.float32)
    rlo = r2.astype(bf16)
    plt = np.ones((64, 4 * n), dtype=bf16)
    prt = np.ones((64, 4 * n), dtype=bf16)
    feats_l = [xb[..., 0], xb[..., 1], rhi, rmid, rlo]
    feats_r = [(xb[..., 0] * bf16(-2.0)), (xb[..., 1] * bf16(-2.0)),
               None, None, None, rhi, rmid, rlo]
    for g in range(gpc):
        g_, gg = divmod(g, 4)
        cols = slice(gg * n, (gg + 1) * n)
        for f, arr in enumerate(feats_l):
            plt[32 * g_ + f, cols] = arr[g]
        for f, arr in enumerate(feats_r):
            if arr is not None:
                prt[32 * g_ + f, cols] = arr[g]
    return plt, prt


def _build_grids(edge_attr):
    """edge_attr [E,2] f32 -> lk bf16 array [NCORES, GPC, PB, 2, TB, N]."""
    tb = N // PB
    ea = edge_attr.astype(bf16).reshape(NUM_GRAPHS, N * (N - 1), 2)
    offdiag = (~np.eye(N, dtype=bool)).reshape(-1)
    grid = np.zeros((2, NUM_GRAPHS, N * N), dtype=bf16)
    grid[0][:, offdiag] = ea[:, :, 0]
    grid[1][:, offdiag] = ea[:, :, 1]
    # [2, graphs, t, p, j] -> [cores, gpc, p, 2, t*j]
    g5 = grid.reshape(2, NUM_GRAPHS, tb, PB, N)
    lk = np.ascontiguousarray(g5.transpose(1, 3, 0, 2, 4))  # [G, PB, 2, tb, N]
    return lk.reshape(NCORES, GPC, PB, 2, tb * N)


def kernel(p, edge_attr, src, dst):
    p = np.ascontiguousarray(np.asarray(p, dtype=np.float32))
    edge_attr = np.ascontiguousarray(np.asarray(edge_attr, dtype=np.float32))
    src = np.asarray(src, dtype=np.int32)
    dst = np.asarray(dst, dtype=np.int32)

    if not _structure_ok(src, dst):
        return _fallback_numpy(p, edge_attr, src, dst)

    from concourse.bass_utils import run_bass_kernel_spmd

    lk = _build_grids(edge_attr)
    pcs = p.reshape(NCORES, GPC * N, 2)

    nc = _get_nc()
    in_maps = []
    for c in range(NCORES):
        plt, prt = _build_plt_prt(pcs[c])
        in_maps.append({"lk": lk[c], "plin": plt, "prin": prt})
    res = run_bass_kernel_spmd(nc, in_maps, list(range(NCORES)))
    total = sum(float(res.results[c]["out"][0, 0]) for c in range(NCORES))
    return np.float32(total)


if __name__ == "__main__":
    nc = _get_nc()
    print("compiled ok")


# revision 43
# speedup vs baseline: 1.1479x; 1.1167x over previous
"""Trainium2 Bass kernel for the all-pairs spring-energy sum (EnergyLossVectorized).

Contract: kernel(**inputs) takes FULL unsharded inputs (p [32768,2] f32,
edge_attr [E,2] f32, src/dst [E] i32 with E = 64*512*511), returns the FULL
scalar output, distributing across 8 NeuronCores internally.

Strategy: src/dst produced by the reference's setup_inputs() are the
deterministic all-directed-pairs (i != j) indices per graph, in i-major
order.  We verify that structure on the host (falling back to a straight
numpy evaluation if it ever doesn't hold) and then compute the energy with
a gather-free formulation:

  For each graph g (512 nodes), the 512x512 grid D2[i,j] = |p_i - p_j|^2 is
  computed on the tensor engine as a K=8 matmul  D2 = PL^T @ PR with
     PL features: [ x,  y,  rhi, rmid, rlo, 1, 1, 1 ]
     PR features: [-2x, -2y, 1,   1,   1,   rhi, rmid, rlo ]
  where r = x^2 + y^2 of the bf16-rounded coords is carried as three bf16
  limbs, so the PSUM result equals |p_i - p_j|^2 to ~fp32 accuracy (no
  cancellation blowup), guaranteeing D2 >= -1e-5 and sqrt(D2+EPS) NaN-free.

  edge_attr (l, k) is re-laid-out on the host into per-graph [512,512] bf16
  grids with k=0 on the diagonal, interleaved [p, {l,k}, t, j] so each
  graph is a single contiguous 1 MB DMA.  Per half-graph tile [128 x 1024]:
     s  = sqrt(D2 + EPS)            (scalar engine, PSUM -> SBUF bf16)
     u  = s - l                     (DVE / GPSIMD)
     e  = (u ^ 2) * k  + row-sum    (DVE scalar_tensor_tensor, accum_out)
  Per-row partials accumulate in parts[128, 16]; the final reduction is one
  tensor_reduce + a [1x1] ones-matmul + 0.5 scale on device; the host sums
  the 8 per-core scalars.

Memory traffic per core: 8 graphs * 1 MB = 8.4 MB bf16 -> ~24 us roofline.
"""

import os
import sys

import numpy as np

for _p in ("/opt/trn_rl_repo", "/root/.axon_site/_ro/trn_rl_repo"):
    if os.path.isdir(_p) and _p not in sys.path:
        sys.path.insert(0, _p)

import ml_dtypes

bf16 = ml_dtypes.bfloat16

NUM_GRAPHS = 64
N = 512                      # nodes per graph
NCORES = 8
GPC = NUM_GRAPHS // NCORES   # graphs per core = 8
PB = 128                     # partition block (i-tile)
EPS = 1e-5                   # sqrt clamp; D2 >= -1e-5 guaranteed by 3-limb r

# per-tile ([128,512], 32 tiles) engine assignment, tuned from HW probes:
# DVE TT bf16 hits 2x mode only at free-dim <= 512 (~380ns); tensor_scalar
# with accum_out runs 4x (~330ns); GPS TT ~1.1us; ACT ~687ns; matmul ~600ns.
# SUB (u=s-l): D=vector, G=gpsimd
# VAR: A = ACT Square(u) then DVE e=u2*k ; B = DVE v=u*k then DVE e=v*u
# RED: P = PE ones-matmul into PSUM acc ; T = DVE tensor_scalar accum
# GPSIMD is banned from the hot loop: it shares an SBUF port with the DVE
# and halves concurrent DVE throughput (measured 415ns -> 1370ns).
# square u2=u*u: D = DVE same-src TT (single-source packing), A = ACT Square
# reduce: P = PE ones-matmul, T = DVE tensor_scalar+accum, A = ACT Copy+accum
VAR_PAT = "DADDADDADADDADDADADDADDADADDADDA"   # D=20, A=12
RED_PAT = "PTPAPTPPPTPAPTPPPTPAPTPPPTPAPTPP"   # P=20, T=8, A=4


def _build_nc(gpc=GPC, n=N, pb=PB, debug=False):
    """Build + compile the per-core Bass program (SPMD, same on all cores)."""
    import concourse.bass as bass
    import concourse.tile as tile
    from concourse import bacc, mybir

    tb = n // pb             # i-tiles per graph (4)
    th = tb // 2             # halves per graph (2), each [pb, 2, n]
    fdt = mybir.dt.float32
    bdt = mybir.dt.bfloat16
    AF = mybir.ActivationFunctionType
    AL = mybir.AluOpType

    nc = bacc.Bacc("TRN2", target_bir_lowering=False, debug=debug,
                   num_devices=NCORES)

    # lk: [graph, partition, {l,k}, t*j] so one graph = 1 contiguous DMA
    lk_d = nc.dram_tensor("lk", [gpc, pb, 2, tb * n], bdt,
                          kind="ExternalInput")
    pl_d = nc.dram_tensor("plin", [64, 4 * n], bdt, kind="ExternalInput")
    pr_d = nc.dram_tensor("prin", [64, 4 * n], bdt, kind="ExternalInput")
    out_d = nc.dram_tensor("out", [1, 1], fdt, kind="ExternalOutput")

    lk = lk_d.ap()

    with tile.TileContext(nc) as tc:
        with (
            tc.tile_pool(name="const", bufs=1) as const,
            tc.tile_pool(name="lkp", bufs=3) as lkp,
            tc.tile_pool(name="work", bufs=6) as work,
            tc.tile_pool(name="psum", bufs=6, space="PSUM") as psum,
            tc.tile_pool(name="accp", bufs=1, space="PSUM") as accp,
        ):
            # host-precomputed matmul operands (see _build_plt_prt)
            plt = const.tile([64, 4 * n], bdt)
            prt = const.tile([64, 4 * n], bdt)
            nc.sync.dma_start(plt[:], pl_d.ap())
            nc.sync.dma_start(prt[:], pr_d.ap())

            ones_col = const.tile([pb, 1], fdt)
            nc.vector.memset(ones_col[:], 1.0)
            ones_bf = const.tile([pb, 1], bdt)
            nc.vector.memset(ones_bf[:], 1.0)
            eps_col = const.tile([pb, 1], fdt)
            nc.vector.memset(eps_col[:], EPS)
            zero_col = const.tile([pb, 1], fdt)
            nc.vector.memset(zero_col[:], 0.0)

            n_tiles = gpc * tb
            n_pe = RED_PAT[:n_tiles].count("P")
            n_ts = n_tiles - n_pe
            parts = const.tile([pb, max(1, n_ts)], fdt)
            acc = accp.tile([1, n], fdt)

            idx = 0
            ts_i = 0
            pe_i = 0
            for g2 in range(gpc // 2):      # 2-graph DMA chunks (2 MB)
                lkt = lkp.tile([pb, 4 * tb * n], bdt)
                nc.sync.dma_start(
                    lkt[:],
                    lk[2 * g2:2 * g2 + 2].rearrange("g p c w -> p g c w"))
                for gg2 in range(2):
                    g = 2 * g2 + gg2
                    g_, gg = divmod(g, 4)
                    for t in range(tb):
                        ps = psum.tile([pb, n], fdt)
                        nc.tensor.matmul(
                            ps[:],
                            plt[32 * g_:32 * g_ + 8,
                                gg * n + t * pb: gg * n + (t + 1) * pb],
                            prt[32 * g_:32 * g_ + 8, gg * n:(gg + 1) * n],
                            start=True, stop=True,
                        )
                        s = work.tile([pb, n], bdt, tag="s")
                        nc.scalar.activation(s[:], ps[:], AF.Sqrt,
                                             bias=eps_col[:])
                        u = work.tile([pb, n], bdt, tag="u")
                        lo = (2 * gg2 + 0) * tb * n + t * n
                        ko = (2 * gg2 + 1) * tb * n + t * n
                        lsl = lkt[:, lo:lo + n]
                        ksl = lkt[:, ko:ko + n]
                        nc.vector.tensor_sub(u[:], s[:], lsl)
                        v = work.tile([pb, n], bdt, tag="v")
                        e = work.tile([pb, n], bdt, tag="e")
                        if VAR_PAT[idx] == "A":
                            nc.scalar.activation(v[:], u[:], AF.Square,
                                                 bias=zero_col[:])
                        else:
                            nc.vector.tensor_mul(v[:], u[:], u[:])
                        nc.vector.tensor_mul(e[:], v[:], ksl)
                        red = RED_PAT[idx]
                        if red == "P":
                            nc.tensor.matmul(
                                acc[:], ones_bf[:], e[:],
                                start=(pe_i == 0), stop=(pe_i == n_pe - 1),
                                skip_group_check=True,
                            )
                            pe_i += 1
                        elif red == "A":
                            e2 = work.tile([pb, n], bdt, tag="e2")
                            nc.scalar.activation(
                                e2[:], e[:], AF.Copy,
                                accum_out=parts[:, ts_i:ts_i + 1])
                            ts_i += 1
                        else:
                            e2 = work.tile([pb, n], bdt, tag="e2")
                            nc.vector.tensor_scalar(
                                e2[:], e[:], 1.0, 0.0, AL.mult, AL.add,
                                accum_out=parts[:, ts_i:ts_i + 1])
                            ts_i += 1
                        idx += 1

            # ---- final reduction to a scalar ----
            # parts [pb, n_ts] -> [pb,1] ; acc [1,n] -> add col-sums
            pr1 = const.tile([pb, 1], fdt)
            nc.vector.tensor_reduce(
                pr1[:], parts[:], axis=mybir.AxisListType.X, op=AL.add)
            acc_sb = const.tile([1, n], fdt)
            nc.vector.tensor_copy(acc_sb[:], acc[:])
            acc1 = const.tile([1, 1], fdt)
            nc.vector.tensor_reduce(
                acc1[:], acc_sb[:], axis=mybir.AxisListType.X, op=AL.add)
            acc11 = accp.tile([1, 1], fdt)
            nc.tensor.matmul(acc11[:], ones_col[:], pr1[:],
                             start=True, stop=True, skip_group_check=True)
            acc11_sb = const.tile([1, 1], fdt)
            nc.vector.tensor_copy(acc11_sb[:], acc11[:])
            tot = const.tile([1, 1], fdt)
            nc.vector.tensor_add(tot[:], acc11_sb[:], acc1[:])
            tot2 = const.tile([1, 1], fdt)
            nc.vector.tensor_scalar_mul(tot2[:], tot[:], 0.5)
            nc.sync.dma_start(out_d.ap(), tot2[:])

    nc.compile()
    return nc


_NC_CACHE = {}


def _get_nc(gpc=GPC, n=N, pb=PB):
    key = (gpc, n, pb)
    if key not in _NC_CACHE:
        _NC_CACHE[key] = _build_nc(gpc, n, pb)
    return _NC_CACHE[key]


def _expected_pairs(num_graphs, n):
    i = np.repeat(np.arange(n, dtype=np.int64), n)
    j = np.tile(np.arange(n, dtype=np.int64), n)
    keep = i != j
    si, sj = i[keep], j[keep]
    off = (np.arange(num_graphs, dtype=np.int64) * n)[:, None]
    src = (off + si[None, :]).reshape(-1)
    dst = (off + sj[None, :]).reshape(-1)
    return src.astype(np.int32), dst.astype(np.int32)


def _structure_ok(src, dst):
    if src.shape != (NUM_GRAPHS * N * (N - 1),):
        return False
    esrc, edst = _expected_pairs(NUM_GRAPHS, N)
    return np.array_equal(src, esrc) and np.array_equal(dst, edst)


def _fallback_numpy(p, edge_attr, src, dst):
    start = p[src].astype(np.float64)
    end = p[dst].astype(np.float64)
    t12 = ((start - end) ** 2).sum(axis=1)
    l = edge_attr[:, 0].astype(np.float64)
    k = edge_attr[:, 1].astype(np.float64)
    energy = k / 2.0 * (t12 + l * l - 2.0 * l * np.sqrt(t12))
    return np.float32(energy.sum())


def _build_plt_prt(p_core, gpc=GPC, n=N):
    """p_core [gpc*n, 2] f32 -> (plt, prt) [64, 4n] bf16 matmul operands."""
    xb = p_core.reshape(gpc, n, 2).astype(bf16)          # bf16-rounded coords
    xf = xb[..., 0].astype(np.float32)
    yf = xb[..., 1].astype(np.float32)
    r = xf * xf + yf * yf
    rhi = r.astype(bf16)
    r1 = r - rhi.astype(np.float32)
    rmid = r1.astype(bf16)
    r2 = r1 - rmid.astype(np.float32)
    rlo = r2.astype(bf16)
    plt = np.ones((64, 4 * n), dtype=bf16)
    prt = np.ones((64, 4 * n), dtype=bf16)
    feats_l = [xb[..., 0], xb[..., 1], rhi, rmid, rlo]
    feats_r = [(xb[..., 0] * bf16(-2.0)), (xb[..., 1] * bf16(-2.0)),
               None, None, None, rhi, rmid, rlo]
    for g in range(gpc):
        g_, gg = divmod(g, 4)
        cols = slice(gg * n, (gg + 1) * n)
        for f, arr in enumerate(feats_l):
            plt[32 * g_ + f, cols] = arr[g]
        for f, arr in enumerate(feats_r):
            if arr is not None:
                prt[32 * g_ + f, cols] = arr[g]
    return plt, prt


def _build_grids(edge_attr):
    """edge_attr [E,2] f32 -> lk bf16 array [NCORES, GPC, PB, 2, TB, N]."""
    tb = N // PB
    ea = edge_attr.astype(bf16).reshape(NUM_GRAPHS, N * (N - 1), 2)
    offdiag = (~np.eye(N, dtype=bool)).reshape(-1)
    grid = np.zeros((2, NUM_GRAPHS, N * N), dtype=bf16)
    grid[0][:, offdiag] = ea[:, :, 0]
    grid[1][:, offdiag] = ea[:, :, 1]
    # [2, graphs, t, p, j] -> [cores, gpc, p, 2, t*j]
    g5 = grid.reshape(2, NUM_GRAPHS, tb, PB, N)
    lk = np.ascontiguousarray(g5.transpose(1, 3, 0, 2, 4))  # [G, PB, 2, tb, N]
    return lk.reshape(NCORES, GPC, PB, 2, tb * N)


def kernel(p, edge_attr, src, dst):
    p = np.ascontiguousarray(np.asarray(p, dtype=np.float32))
    edge_attr = np.ascontiguousarray(np.asarray(edge_attr, dtype=np.float32))
    src = np.asarray(src, dtype=np.int32)
    dst = np.asarray(dst, dtype=np.int32)

    if not _structure_ok(src, dst):
        return _fallback_numpy(p, edge_attr, src, dst)

    from concourse.bass_utils import run_bass_kernel_spmd

    lk = _build_grids(edge_attr)
    pcs = p.reshape(NCORES, GPC * N, 2)

    nc = _get_nc()
    in_maps = []
    for c in range(NCORES):
        plt, prt = _build_plt_prt(pcs[c])
        in_maps.append({"lk": lk[c], "plin": plt, "prin": prt})
    res = run_bass_kernel_spmd(nc, in_maps, list(range(NCORES)))
    total = sum(float(res.results[c]["out"][0, 0]) for c in range(NCORES))
    return np.float32(total)


if __name__ == "__main__":
    nc = _get_nc()
    print("compiled ok")


# revision 46
# speedup vs baseline: 1.2398x; 1.0801x over previous
"""Trainium2 Bass kernel for the all-pairs spring-energy sum (EnergyLossVectorized).

Contract: kernel(**inputs) takes FULL unsharded inputs (p [32768,2] f32,
edge_attr [E,2] f32, src/dst [E] i32 with E = 64*512*511), returns the FULL
scalar output, distributing across 8 NeuronCores internally.

Strategy: src/dst produced by the reference's setup_inputs() are the
deterministic all-directed-pairs (i != j) indices per graph, in i-major
order.  We verify that structure on the host (falling back to a straight
numpy evaluation if it ever doesn't hold) and then compute the energy with
a gather-free formulation:

  For each graph g (512 nodes), the 512x512 grid D2[i,j] = |p_i - p_j|^2 is
  computed on the tensor engine as a K=8 matmul  D2 = PL^T @ PR with
     PL features: [ x,  y,  rhi, rmid, rlo, 1, 1, 1 ]
     PR features: [-2x, -2y, 1,   1,   1,   rhi, rmid, rlo ]
  where r = x^2 + y^2 of the bf16-rounded coords is carried as three bf16
  limbs, so the PSUM result equals |p_i - p_j|^2 to ~fp32 accuracy (no
  cancellation blowup), guaranteeing D2 >= -1e-5 and sqrt(D2+EPS) NaN-free.

  edge_attr (l, k) is re-laid-out on the host into per-graph [512,512] bf16
  grids with k=0 on the diagonal, interleaved [p, {l,k}, t, j] so each
  graph is a single contiguous 1 MB DMA.  Per half-graph tile [128 x 1024]:
     s  = sqrt(D2 + EPS)            (scalar engine, PSUM -> SBUF bf16)
     u  = s - l                     (DVE / GPSIMD)
     e  = (u ^ 2) * k  + row-sum    (DVE scalar_tensor_tensor, accum_out)
  Per-row partials accumulate in parts[128, 16]; the final reduction is one
  tensor_reduce + a [1x1] ones-matmul + 0.5 scale on device; the host sums
  the 8 per-core scalars.

Memory traffic per core: 8 graphs * 1 MB = 8.4 MB bf16 -> ~24 us roofline.
"""

import os
import sys

import numpy as np

for _p in ("/opt/trn_rl_repo", "/root/.axon_site/_ro/trn_rl_repo"):
    if os.path.isdir(_p) and _p not in sys.path:
        sys.path.insert(0, _p)

import ml_dtypes

bf16 = ml_dtypes.bfloat16

NUM_GRAPHS = 64
N = 512                      # nodes per graph
NCORES = 8
GPC = NUM_GRAPHS // NCORES   # graphs per core = 8
PB = 128                     # partition block (i-tile)
EPS = 1e-5                   # sqrt clamp; D2 >= -1e-5 guaranteed by 3-limb r

# per-tile ([128,512], 32 tiles) engine assignment, tuned from HW probes:
# DVE TT bf16 hits 2x mode only at free-dim <= 512 (~380ns); tensor_scalar
# with accum_out runs 4x (~330ns); GPS TT ~1.1us; ACT ~687ns; matmul ~600ns.
# SUB (u=s-l): D=vector, G=gpsimd
# VAR: A = ACT Square(u) then DVE e=u2*k ; B = DVE v=u*k then DVE e=v*u
# RED: P = PE ones-matmul into PSUM acc ; T = DVE tensor_scalar accum
# GPSIMD is banned from the hot loop: it shares an SBUF port with the DVE
# and halves concurrent DVE throughput (measured 415ns -> 1370ns).
# ACT ops are batched per PAIR of t-tiles ([128,1024]) to amortize the
# ~224-cycle ACT fixed cost; DVE ops stay at [128,512] (2x-mode limit).
# per-pair square: D = 2x DVE same-src TT, A = one ACT Square [128,1024]
# per-pair reduce: P = 2x PE ones-matmul, T = one DVE TS+accum [128,1024],
#                  A = one ACT Copy+accum [128,1024]
VARP_PAT = "DADADADADADADADA"   # per pair: D=8, A=8
REDP_PAT = "PPAPTPAPPTAPPATP"   # per pair: P=9, A=4, T=3


def _build_nc(gpc=GPC, n=N, pb=PB, debug=False):
    """Build + compile the per-core Bass program (SPMD, same on all cores)."""
    import concourse.bass as bass
    import concourse.tile as tile
    from concourse import bacc, mybir

    tb = n // pb             # i-tiles per graph (4)
    th = tb // 2             # halves per graph (2), each [pb, 2, n]
    fdt = mybir.dt.float32
    bdt = mybir.dt.bfloat16
    AF = mybir.ActivationFunctionType
    AL = mybir.AluOpType

    nc = bacc.Bacc("TRN2", target_bir_lowering=False, debug=debug,
                   num_devices=NCORES)

    # lk: [graph, partition, {l,k}, t*j] so one graph = 1 contiguous DMA
    lk_d = nc.dram_tensor("lk", [gpc, pb, 2, tb * n], bdt,
                          kind="ExternalInput")
    pl_d = nc.dram_tensor("plin", [64, 4 * n], bdt, kind="ExternalInput")
    pr_d = nc.dram_tensor("prin", [64, 4 * n], bdt, kind="ExternalInput")
    out_d = nc.dram_tensor("out", [1, 1], fdt, kind="ExternalOutput")

    lk = lk_d.ap()

    with tile.TileContext(nc) as tc:
        with (
            tc.tile_pool(name="const", bufs=1) as const,
            tc.tile_pool(name="lkp", bufs=3) as lkp,
            tc.tile_pool(name="work", bufs=6) as work,
            tc.tile_pool(name="psum", bufs=3, space="PSUM") as psum,
            tc.tile_pool(name="accp", bufs=1, space="PSUM") as accp,
        ):
            # host-precomputed matmul operands (see _build_plt_prt)
            plt = const.tile([64, 4 * n], bdt)
            prt = const.tile([64, 4 * n], bdt)
            nc.sync.dma_start(plt[:], pl_d.ap())
            nc.sync.dma_start(prt[:], pr_d.ap())

            ones_col = const.tile([pb, 1], fdt)
            nc.vector.memset(ones_col[:], 1.0)
            ones_bf = const.tile([pb, 1], bdt)
            nc.vector.memset(ones_bf[:], 1.0)
            eps_col = const.tile([pb, 1], fdt)
            nc.vector.memset(eps_col[:], EPS)
            zero_col = const.tile([pb, 1], fdt)
            nc.vector.memset(zero_col[:], 0.0)

            n_pairs = gpc * th
            n_pe = 2 * REDP_PAT[:n_pairs].count("P")
            n_ts = n_pairs - REDP_PAT[:n_pairs].count("P")
            parts = const.tile([pb, max(1, n_ts)], fdt)
            acc = accp.tile([1, n], fdt)

            pidx = 0
            ts_i = 0
            pe_i = 0
            for g2 in range(gpc // 2):      # 2-graph DMA chunks (2 MB)
                lkt = lkp.tile([pb, 4 * tb * n], bdt)
                nc.sync.dma_start(
                    lkt[:],
                    lk[2 * g2:2 * g2 + 2].rearrange("g p c w -> p g c w"))
                for gg2 in range(2):
                    g = 2 * g2 + gg2
                    g_, gg = divmod(g, 4)
                    for h in range(th):
                        ps = psum.tile([pb, 2 * n], fdt)
                        for tt in range(2):
                            t = 2 * h + tt
                            nc.tensor.matmul(
                                ps[:, tt * n:(tt + 1) * n],
                                plt[32 * g_:32 * g_ + 8,
                                    gg * n + t * pb: gg * n + (t + 1) * pb],
                                prt[32 * g_:32 * g_ + 8, gg * n:(gg + 1) * n],
                                start=True, stop=True,
                            )
                        s = work.tile([pb, 2 * n], bdt, tag="s")
                        nc.scalar.activation(s[:], ps[:], AF.Sqrt,
                                             bias=eps_col[:])
                        u = work.tile([pb, 2 * n], bdt, tag="u")
                        v = work.tile([pb, 2 * n], bdt, tag="v")
                        e = work.tile([pb, 2 * n], bdt, tag="e")
                        base = 2 * gg2 * tb * n + 2 * h * n
                        kbase = base + tb * n
                        for tt in range(2):
                            sl = slice(tt * n, (tt + 1) * n)
                            lsl = lkt[:, base + tt * n: base + (tt + 1) * n]
                            nc.vector.tensor_sub(u[:, sl], s[:, sl], lsl)
                        if VARP_PAT[pidx] == "A":
                            nc.scalar.activation(v[:], u[:], AF.Square,
                                                 bias=zero_col[:])
                        else:
                            for tt in range(2):
                                sl = slice(tt * n, (tt + 1) * n)
                                nc.vector.tensor_mul(v[:, sl], u[:, sl],
                                                     u[:, sl])
                        for tt in range(2):
                            sl = slice(tt * n, (tt + 1) * n)
                            ksl = lkt[:, kbase + tt * n: kbase + (tt + 1) * n]
                            nc.vector.tensor_mul(e[:, sl], v[:, sl], ksl)
                        red = REDP_PAT[pidx]
                        if red == "P":
                            for tt in range(2):
                                sl = slice(tt * n, (tt + 1) * n)
                                nc.tensor.matmul(
                                    acc[:], ones_bf[:], e[:, sl],
                                    start=(pe_i == 0),
                                    stop=(pe_i == n_pe - 1),
                                    skip_group_check=True,
                                )
                                pe_i += 1
                        elif red == "A":
                            e2 = work.tile([pb, 2 * n], bdt, tag="e2")
                            nc.scalar.activation(
                                e2[:], e[:], AF.Copy,
                                accum_out=parts[:, ts_i:ts_i + 1])
                            ts_i += 1
                        else:
                            e2 = work.tile([pb, 2 * n], bdt, tag="e2")
                            nc.vector.tensor_scalar(
                                e2[:], e[:], 1.0, 0.0, AL.mult, AL.add,
                                accum_out=parts[:, ts_i:ts_i + 1])
                            ts_i += 1
                        pidx += 1

            # ---- final reduction to a scalar ----
            # parts [pb, n_ts] -> [pb,1] ; acc [1,n] -> add col-sums
            pr1 = const.tile([pb, 1], fdt)
            nc.vector.tensor_reduce(
                pr1[:], parts[:], axis=mybir.AxisListType.X, op=AL.add)
            acc_sb = const.tile([1, n], fdt)
            nc.vector.tensor_copy(acc_sb[:], acc[:])
            acc1 = const.tile([1, 1], fdt)
            nc.vector.tensor_reduce(
                acc1[:], acc_sb[:], axis=mybir.AxisListType.X, op=AL.add)
            acc11 = accp.tile([1, 1], fdt)
            nc.tensor.matmul(acc11[:], ones_col[:], pr1[:],
                             start=True, stop=True, skip_group_check=True)
            acc11_sb = const.tile([1, 1], fdt)
            nc.vector.tensor_copy(acc11_sb[:], acc11[:])
            tot = const.tile([1, 1], fdt)
            nc.vector.tensor_add(tot[:], acc11_sb[:], acc1[:])
            tot2 = const.tile([1, 1], fdt)
            nc.vector.tensor_scalar_mul(tot2[:], tot[:], 0.5)
            nc.sync.dma_start(out_d.ap(), tot2[:])

    nc.compile()
    return nc


_NC_CACHE = {}


def _get_nc(gpc=GPC, n=N, pb=PB):
    key = (gpc, n, pb)
    if key not in _NC_CACHE:
        _NC_CACHE[key] = _build_nc(gpc, n, pb)
    return _NC_CACHE[key]


def _expected_pairs(num_graphs, n):
    i = np.repeat(np.arange(n, dtype=np.int64), n)
    j = np.tile(np.arange(n, dtype=np.int64), n)
    keep = i != j
    si, sj = i[keep], j[keep]
    off = (np.arange(num_graphs, dtype=np.int64) * n)[:, None]
    src = (off + si[None, :]).reshape(-1)
    dst = (off + sj[None, :]).reshape(-1)
    return src.astype(np.int32), dst.astype(np.int32)


def _structure_ok(src, dst):
    if src.shape != (NUM_GRAPHS * N * (N - 1),):
        return False
    esrc, edst = _expected_pairs(NUM_GRAPHS, N)
    return np.array_equal(src, esrc) and np.array_equal(dst, edst)


def _fallback_numpy(p, edge_attr, src, dst):
    start = p[src].astype(np.float64)
    end = p[dst].astype(np.float64)
    t12 = ((start - end) ** 2).sum(axis=1)
    l = edge_attr[:, 0].astype(np.float64)
    k = edge_attr[:, 1].astype(np.float64)
    energy = k / 2.0 * (t12 + l * l - 2.0 * l * np.sqrt(t12))
    return np.float32(energy.sum())


def _build_plt_prt(p_core, gpc=GPC, n=N):
    """p_core [gpc*n, 2] f32 -> (plt, prt) [64, 4n] bf16 matmul operands."""
    xb = p_core.reshape(gpc, n, 2).astype(bf16)          # bf16-rounded coords
    xf = xb[..., 0].astype(np.float32)
    yf = xb[..., 1].astype(np.float32)
    r = xf * xf + yf * yf
    rhi = r.astype(bf16)
    r1 = r - rhi.astype(np.float32)
    rmid = r1.astype(bf16)
    r2 = r1 - rmid.astype(np.float32)
    rlo = r2.astype(bf16)
    plt = np.ones((64, 4 * n), dtype=bf16)
    prt = np.ones((64, 4 * n), dtype=bf16)
    feats_l = [xb[..., 0], xb[..., 1], rhi, rmid, rlo]
    feats_r = [(xb[..., 0] * bf16(-2.0)), (xb[..., 1] * bf16(-2.0)),
               None, None, None, rhi, rmid, rlo]
    for g in range(gpc):
        g_, gg = divmod(g, 4)
        cols = slice(gg * n, (gg + 1) * n)
        for f, arr in enumerate(feats_l):
            plt[32 * g_ + f, cols] = arr[g]
        for f, arr in enumerate(feats_r):
            if arr is not None:
                prt[32 * g_ + f, cols] = arr[g]
    return plt, prt


def _build_grids(edge_attr):
    """edge_attr [E,2] f32 -> lk bf16 array [NCORES, GPC, PB, 2, TB, N]."""
    tb = N // PB
    ea = edge_attr.astype(bf16).reshape(NUM_GRAPHS, N * (N - 1), 2)
    offdiag = (~np.eye(N, dtype=bool)).reshape(-1)
    grid = np.zeros((2, NUM_GRAPHS, N * N), dtype=bf16)
    grid[0][:, offdiag] = ea[:, :, 0]
    grid[1][:, offdiag] = ea[:, :, 1]
    # [2, graphs, t, p, j] -> [cores, gpc, p, 2, t*j]
    g5 = grid.reshape(2, NUM_GRAPHS, tb, PB, N)
    lk = np.ascontiguousarray(g5.transpose(1, 3, 0, 2, 4))  # [G, PB, 2, tb, N]
    return lk.reshape(NCORES, GPC, PB, 2, tb * N)


def kernel(p, edge_attr, src, dst):
    p = np.ascontiguousarray(np.asarray(p, dtype=np.float32))
    edge_attr = np.ascontiguousarray(np.asarray(edge_attr, dtype=np.float32))
    src = np.asarray(src, dtype=np.int32)
    dst = np.asarray(dst, dtype=np.int32)

    if not _structure_ok(src, dst):
        return _fallback_numpy(p, edge_attr, src, dst)

    from concourse.bass_utils import run_bass_kernel_spmd

    lk = _build_grids(edge_attr)
    pcs = p.reshape(NCORES, GPC * N, 2)

    nc = _get_nc()
    in_maps = []
    for c in range(NCORES):
        plt, prt = _build_plt_prt(pcs[c])
        in_maps.append({"lk": lk[c], "plin": plt, "prin": prt})
    res = run_bass_kernel_spmd(nc, in_maps, list(range(NCORES)))
    total = sum(float(res.results[c]["out"][0, 0]) for c in range(NCORES))
    return np.float32(total)


if __name__ == "__main__":
    nc = _get_nc()
    print("compiled ok")
